# revision 1
# baseline (speedup 1.0000x reference)
"""Trainium2 Bass kernel for nn_CustomLSTM: B=32, S=512, D_in=512, D_h=1024, D_out=512.

Strategy: 8-way tensor-parallel over the hidden/gate dim. Core c owns 128 h-dims
(block c) and the 4x128 = 512 gate columns that produce them. Per step:
  - PE: gates_preact[32,512] = bias + x_t @ Wx_loc + h_t @ Wh_loc  (fp32r, psum)
  - ACT: sigmoid(f,i,o) + tanh(g)  -> SBUF (untransposed [32,512])
  - PE: 4 transposes -> gates^T [128,32] each (psum)
  - DVE: c' = f*c + i*g ; h^T = o * tanh(c')  (transposed [128,32])
  - h^T tile [128,32] AllGather'd across the 8 cores (collective_compute via
    DRAM bounce buffers), giving every core the full h_{t+1}^T [1024,32].
FC (hs @ W_fc + b_fc) runs as an epilogue: every step the gathered h^T [128,256]
is saved to DRAM; afterwards each core computes the output for its own 64-step
window (window start = partition_id * 64, via a register-offset DMA).
"""

import sys

if "/opt/trn_rl_repo" not in sys.path:
    sys.path.insert(0, "/opt/trn_rl_repo")

import numpy as np

B, S, DIN, DH, DOUT = 32, 512, 512, 1024, 512
NCORES = 8
LOCH = DH // NCORES          # 128 h-dims per core
LOCG = 4 * LOCH              # 512 gate cols per core (f|i|o|g)
KX = DIN // 128              # 4 x k-tiles
KH = 8                       # 8 h slot tiles
XT_RING = 8                  # xt prefetch ring depth (steps)
XT_AHEAD = 6                 # prefetch distance

_cache = {}


def _build_nc(T):
    """Build the SPMD bass program for a T-step LSTM (T must be divisible by 8)."""
    from concourse import bass
    import concourse.mybir as mybir

    dt = mybir.dt
    f32 = dt.float32
    f32r = dt.float32r
    AF = mybir.ActivationFunctionType

    TWIN = T // NCORES  # FC window per core
    nc = bass.Bass(target_bir_lowering=False, num_devices=NCORES)

    # ---------------- I/O ----------------
    xT = nc.dram_tensor("xT", [DIN, T, B], f32, kind="ExternalInput")
    wxin = nc.dram_tensor("wxin", [DIN, LOCG], f32, kind="ExternalInput")
    whin = nc.dram_tensor("whin", [DH, LOCG], f32, kind="ExternalInput")
    wfcin = nc.dram_tensor("wfcin", [DH, DOUT], f32, kind="ExternalInput")
    bin_ = nc.dram_tensor("bin", [1, LOCG], f32, kind="ExternalInput")
    bfcin = nc.dram_tensor("bfcin", [1, DOUT], f32, kind="ExternalInput")
    onesin = nc.dram_tensor("onesin", [1, 256], f32, kind="ExternalInput")
    identin = nc.dram_tensor("identin", [128, 128], f32, kind="ExternalInput")
    zeroin = nc.dram_tensor("zeroin", [128, 256], f32, kind="ExternalInput")
    outT = nc.dram_tensor("outT", [DOUT, TWIN * B], f32, kind="ExternalOutput")
    # gathered h^T history: [128 part, T * 256] (256 = 8 slots x 32 batch)
    hbuf = nc.dram_tensor("hbuf", [128, T * 256], f32)
    # collective bounce buffers (double-buffered by step parity)
    bci = nc.dram_tensor("bci", [2, 128, B], f32)
    bco = nc.dram_tensor("bco", [2, KH * 128, B], f32, addr_space="Shared")
    RG = [list(range(NCORES))]

    # ---------------- semaphores ----------------
    s_ld = nc.alloc_semaphore("s_ld")        # prologue dma loads (+16)
    s_xt = nc.alloc_semaphore("s_xt")        # xt prefetch dmas (+16)
    s_hrdy = nc.alloc_semaphore("s_hrdy")    # DVE h^T writes (+1/step)
    s_mm = nc.alloc_semaphore("s_mm")        # PE last gate-MM (+1/step)
    s_act = nc.alloc_semaphore("s_act")      # ACT sig+tanh done (+1/step)
    s_T = nc.alloc_semaphore("s_T")          # PE transposes done (+1/step)
    s_cp = nc.alloc_semaphore("s_cp")        # DVE c' written (+1/step)
    s_tc = nc.alloc_semaphore("s_tc")        # ACT tanh(c) written (+1/step)
    s_save = nc.alloc_semaphore("s_save")    # hbuf save dmas (+16/step)
    s_bi = nc.alloc_semaphore("s_bi")        # bounce-in dmas (+16/step)
    s_big = nc.alloc_semaphore("s_big")      # bounce-in confirmed (+1/step)
    s_cc = nc.alloc_semaphore("s_cc")        # collectives done (+1/step)
    s_fill = nc.alloc_semaphore("s_fill")    # gath fill dmas (+16/step)
    s_hing = nc.alloc_semaphore("s_hing")    # gath fill confirmed (+1/step)
    s_fcl = nc.alloc_semaphore("s_fcl")      # FC hwin loads (+16)
    s_fcm = nc.alloc_semaphore("s_fcm")      # FC psum group done (+1)
    s_fce = nc.alloc_semaphore("s_fce")      # FC evac done (+1)
    s_fout = nc.alloc_semaphore("s_fout")    # FC out dmas (+16)
    s_xtg = nc.alloc_semaphore("s_xtg")      # xt groups confirmed (+1)
    s_saveg = nc.alloc_semaphore("s_saveg")  # saves confirmed (+1)
    s_dv1 = nc.alloc_semaphore("s_dv1")      # DVE writeback fences (+2/step)
    local_sems = [s_ld, s_xt, s_hrdy, s_mm, s_act, s_T, s_cp, s_tc, s_save,
                  s_bi, s_big, s_cc, s_fill, s_hing, s_fcl, s_fcm, s_fce,
                  s_fout, s_xtg, s_saveg, s_dv1]
    all_sems = local_sems

    # ---------------- on-chip tensors ----------------
    ctx_tensors = []

    def sbuf(name, shape, dtype=f32):
        cm = nc.sbuf_tensor(name, shape, dtype)
        t = cm.__enter__()
        ctx_tensors.append(cm)
        return t

    def psum(name, shape, dtype=f32):
        cm = nc.psum_tensor(name, shape, dtype)
        t = cm.__enter__()
        ctx_tensors.append(cm)
        return t

    wx = sbuf("wx", [128, KX * LOCG])          # x-weights, tile d at cols d*512
    wh = sbuf("wh", [128, KH * LOCG])          # h-weights, slot tile m at m*512
    wfc = sbuf("wfc", [128, KH * DOUT])        # fc weights, tile (m,dout)
    xt = sbuf("xt", [128, XT_RING * KX * B])   # x_t^T ring: slot r at cols r*128
    gath = sbuf("gath", [128, 2 * 256])        # gathered h^T, 2 parities x 8 slots
    actsb = sbuf("actsb", [128, 2 * LOCG])     # activated gates (rows 0:32)
    csb = sbuf("csb", [128, 2 * B])            # c^T state, 2 parities
    t1b = sbuf("t1b", [128, B])
    t2b = sbuf("t2b", [128, B])
    gsb = sbuf("gsb", [128, B])
    tcb = sbuf("tcb", [128, B])
    hst = sbuf("hst", [128, B])                # h^T staging for the collective
    bvec = sbuf("bvec", [1, LOCG])
    bfcv = sbuf("bfcv", [1, DOUT])
    onesb = sbuf("onesb", [1, 256])
    ident = sbuf("ident", [128, 128])
    FCCH = min(256, TWIN * B)                  # FC matmul chunk (cols)
    NCH = (TWIN * B) // FCCH                   # chunks per m-tile
    hwin = sbuf("hwin", [128, KH * TWIN * B])  # FC window, slot-major
    fco = sbuf("fco", [128, 2 * TWIN * B])     # FC out staging, parity by m-tile

    pg = psum("pg", [128, 2 * 512])            # gate psum, 2 banks (rows 0:32)
    pt = psum("pt", [128, 2 * 512])            # transposed gates, 2 banks
    pfc = psum("pfc", [128, 2 * 512])          # FC psum, 2 banks

    def r(ap):
        return ap.bitcast(f32r)

    # ================= Block P0: clear sems =================
    with nc.Block() as blk:
        @blk.gpsimd
        def _(gp):
            for sm in local_sems:
                gp.sem_clear(sm)

    # ================= Block P1: load weights / init state =================
    with nc.Block() as blk:
        @blk.sync
        def _(sp):
            n = 0
            for d in range(KX):
                sp.dma_start(r(wx[:, 512 * d:512 * (d + 1)]),
                             r(wxin[128 * d:128 * (d + 1), :])).then_inc(s_ld, 16)
                n += 1
            for m in range(KH):
                sp.dma_start(r(wh[:, 512 * m:512 * (m + 1)]),
                             r(whin[128 * m:128 * (m + 1), :])).then_inc(s_ld, 16)
                n += 1
            for m in range(KH):
                sp.dma_start(r(wfc[:, 512 * m:512 * (m + 1)]),
                             r(wfcin[128 * m:128 * (m + 1), :])).then_inc(s_ld, 16)
                n += 1
            P = min(XT_AHEAD, T)
            for t0 in range(P):
                for d in range(KX):
                    sp.dma_start(r(xt[:, 128 * t0 + 32 * d:128 * t0 + 32 * (d + 1)]),
                                 r(xT[128 * d:128 * (d + 1), t0, :])).then_inc(s_xt, 16)
            sp.wait_ge(s_xt, 16 * KX * P)
            sp.sem_inc(s_xtg, P)
            sp.dma_start(r(bvec[0:1, :]), r(bin_[0:1, :])).then_inc(s_ld, 16); n += 1
            sp.dma_start(r(bfcv[0:1, :]), r(bfcin[0:1, :])).then_inc(s_ld, 16); n += 1
            sp.dma_start(r(onesb[0:1, :]), r(onesin[0:1, :])).then_inc(s_ld, 16); n += 1
            sp.dma_start(ident[:, :], identin[:, :]).then_inc(s_ld, 16); n += 1
            sp.dma_start(r(gath[:, 0:256]), r(zeroin[:, :])).then_inc(s_ld, 16)
            n += 1  # h_0 = 0 (parity 0)
            sp.wait_ge(s_ld, 16 * n)

        @blk.vector
        def _(ve):
            ve.memset(csb[:, 0:B], 0.0)         # c_0 = 0

    # ================= Block M: the recurrence =================
    with nc.Block() as blk:
        @blk.tensor
        def _(te):
            for t in range(T):
                pi = t % 2
                pgb = pg[0:32, 512 * pi:512 * (pi + 1)]        # gate psum bank
                if t >= 2:
                    te.wait_ge(s_act, t - 1)                   # pg[pi] free
                te.wait_ge(s_xtg, t + 1)                       # xt(t) loaded
                te.matmul(pgb, r(onesb[0:1, 0:32]), r(bvec[0:1, :]),
                          start=True, stop=False)
                xs = 128 * (t % XT_RING)
                for d in range(KX):
                    te.matmul(pgb, r(xt[:, xs + 32 * d:xs + 32 * (d + 1)]),
                              r(wx[:, 512 * d:512 * (d + 1)]),
                              start=False, stop=False)
                if t >= 1:
                    te.wait_ge(s_hing, t)                      # h_t gathered
                gb = 256 * pi
                for m in range(KH):
                    mm = te.matmul(pgb, r(gath[:, gb + 32 * m:gb + 32 * (m + 1)]),
                                   r(wh[:, 512 * m:512 * (m + 1)]),
                                   start=False, stop=(m == KH - 1))
                    if m == KH - 1:
                        mm.then_inc(s_mm, 1)
                # transposes of activated gates -> pt[pi] cols 0:128 (F|I|O|G)
                te.wait_ge(s_act, t + 1)
                ab = 512 * pi
                ptb = 512 * pi
                for gi in range(4):
                    mm = te.matmul(pt[:, ptb + 32 * gi:ptb + 32 * (gi + 1)],
                                   actsb[0:32, ab + 128 * gi:ab + 128 * (gi + 1)],
                                   ident[0:32, 0:32],
                                   is_transpose=True, start=True, stop=True,
                                   skip_group_check=True)
                    if gi == 3:
                        mm.then_inc(s_T, 1)

        @blk.scalar
        def _(ac):
            for t in range(T):
                pi = t % 2
                ab = 512 * pi
                ac.wait_ge(s_mm, t + 1)
                if t >= 2:
                    ac.wait_ge(s_T, t - 1)                     # actsb[pi] free
                ac.activation(actsb[0:32, ab:ab + 384],
                              pg[0:32, 512 * pi:512 * pi + 384], AF.Sigmoid)
                ac.activation(actsb[0:32, ab + 384:ab + 512],
                              pg[0:32, 512 * pi + 384:512 * pi + 512],
                              AF.Tanh).then_inc(s_act, 1)
                ac.wait_ge(s_cp, t + 1)
                if t >= 1:
                    ac.wait_ge(s_hrdy, t)                      # tcb free
                cpb = ((t + 1) % 2) * B
                ac.activation(tcb[:, 0:B], csb[:, cpb:cpb + B],
                              AF.Tanh).then_inc(s_tc, 1)

        @blk.vector
        def _(ve):
            for t in range(T):
                pi = t % 2
                po = (t + 1) % 2
                ptb = 512 * pi
                F = pt[:, ptb + 0:ptb + 32]
                I = pt[:, ptb + 32:ptb + 64]
                O = pt[:, ptb + 64:ptb + 96]
                G = pt[:, ptb + 96:ptb + 128]
                ve.wait_ge(s_T, t + 1)
                ve.tensor_copy(gsb[:, :], G).then_inc(s_dv1, 1)  # PSUM -> SBUF
                ve.tensor_mul(t1b[:, :], F, csb[:, pi * B:pi * B + B])
                ve.wait_ge(s_dv1, 2 * t + 1)                   # gsb writeback fence
                ve.tensor_mul(t2b[:, :], I, gsb[:, :]).then_inc(s_dv1, 1)
                if t >= 2:
                    ve.wait_ge(s_tc, t - 1)                    # csb[po] free
                ve.wait_ge(s_dv1, 2 * t + 2)                   # t1/t2 writeback fence
                ve.tensor_add(csb[:, po * B:po * B + B], t1b[:, :],
                              t2b[:, :]).then_inc(s_cp, 1)
                ve.wait_ge(s_tc, t + 1)
                if t >= 1:
                    ve.wait_ge(s_big, t)                       # bounce-in(t-1) done
                ve.tensor_mul(r(hst[:, :]), O, tcb[:, 0:B]).then_inc(s_hrdy, 1)

        @blk.sync
        def _(sp):
            nxt = 16 * KX * min(XT_AHEAD, T)
            for t in range(T):
                po = (t + 1) % 2
                # prefetch x^T for t+XT_AHEAD
                tf = t + XT_AHEAD
                if tf < T:
                    if t >= 2:
                        sp.wait_ge(s_mm, t - 1)                # ring slot free
                    xs = 128 * (tf % XT_RING)
                    for d in range(KX):
                        sp.dma_start(r(xt[:, xs + 32 * d:xs + 32 * (d + 1)]),
                                     r(xT[128 * d:128 * (d + 1), tf, :])).then_inc(
                                         s_xt, 16)
                    nxt += 16 * KX
                # stage h_{t+1} into bounce-in
                sp.wait_ge(s_hrdy, t + 1)
                if t >= 2:
                    sp.wait_ge(s_cc, t - 1)                    # cc(t-2) done w/ bci[po]
                sp.dma_start(r(bci[po]), r(hst[:, :])).then_inc(s_bi, 16)
                sp.wait_ge(s_bi, 16 * (t + 1))
                sp.sem_inc(s_big, 1)
                if tf < T:
                    sp.wait_ge(s_xt, nxt)
                    sp.sem_inc(s_xtg, 1)
                # fill gath[po] from the collective output
                sp.wait_ge(s_cc, t + 1)
                sp.wait_ge(s_mm, t)                            # PE(t-1) done w/ gath[po]
                sp.dma_start(
                    r(gath[:, 256 * po:256 * (po + 1)]),
                    r(bco[po].rearrange("(m p) b -> p m b", p=128))).then_inc(
                        s_fill, 16)
                sp.wait_ge(s_fill, 16 * (t + 1))
                sp.sem_inc(s_hing, 1)
                # save gathered h_{t+1} for the FC epilogue
                sp.dma_start(r(hbuf[:, 256 * t:256 * (t + 1)]),
                             r(gath[:, 256 * po:256 * (po + 1)])).then_inc(s_save, 16)
                sp.wait_ge(s_save, 16 * (t + 1))
                sp.sem_inc(s_saveg, 1)

        @blk.gpsimd
        def _(gp):
            import concourse.mybir as mybir
            for t in range(T):
                po = (t + 1) % 2
                gp.wait_ge(s_bi, 16 * (t + 1))
                gp.collective_compute(
                    "AllGather", mybir.AluOpType.bypass,
                    replica_groups=RG,
                    ins=[bci[po].opt()],
                    outs=[bco[po].opt()]).then_inc(s_cc, 1)

    # ================= Block F: FC epilogue =================
    with nc.Block() as blk:
        @blk.sync
        def _(sp):
            sp.wait_ge(s_saveg, T)                             # all h history saved
            pid = sp.partition_id()
            base = pid * (TWIN * 256)
            hb = hbuf[:, bass.DynSlice(base, TWIN * 256)]
            hb3 = hb.rearrange("p (t c) -> p t c", c=256)
            for sl in range(KH):
                sp.dma_start(
                    r(hwin[:, (TWIN * B) * sl:(TWIN * B) * (sl + 1)]),
                    r(hb3[:, :, 32 * sl:32 * (sl + 1)])).then_inc(s_fcl, 16)
            # output writes (serialized so s_fout rests at multiples of 16)
            for m in range(4):
                sp.wait_ge(s_fce, NCH * (m + 1))
                fp = (m % 2) * (TWIN * B)
                sp.dma_start(outT[128 * m:128 * (m + 1), :],
                             fco[:, fp:fp + TWIN * B]).then_inc(s_fout, 16)
                sp.wait_ge(s_fout, 16 * (m + 1))

        @blk.tensor
        def _(te):
            te.wait_ge(s_fcl, 16 * KH)
            q = 0
            for m in range(4):
                for n in range(NCH):
                    bk = pfc[:, 512 * (q % 2):512 * (q % 2) + FCCH]
                    if q >= 2:
                        te.wait_ge(s_fce, q - 1)
                    te.matmul(bk, r(bfcv[0:1, 128 * m:128 * (m + 1)]),
                              r(onesb[0:1, 0:FCCH]), start=True, stop=False)
                    for sl in range(KH):
                        mm = te.matmul(
                            bk, r(wfc[:, 512 * sl + 128 * m:512 * sl + 128 * (m + 1)]),
                            r(hwin[:, (TWIN * B) * sl + FCCH * n:
                                   (TWIN * B) * sl + FCCH * (n + 1)]),
                            start=False, stop=(sl == KH - 1))
                        if sl == KH - 1:
                            mm.then_inc(s_fcm, 1)
                    q += 1

        @blk.vector
        def _(ve):
            q = 0
            for m in range(4):
                fp = (m % 2) * (TWIN * B)
                if m >= 2:
                    ve.wait_ge(s_fout, 16 * (m - 1))           # fco parity free
                for n in range(NCH):
                    ve.wait_ge(s_fcm, q + 1)
                    ve.tensor_copy(fco[:, fp + FCCH * n:fp + FCCH * (n + 1)],
                                   pfc[:, 512 * (q % 2):512 * (q % 2) + FCCH]).then_inc(
                                       s_fce, 1)
                    q += 1

    # ================= Block E: final cleanup =================
    with nc.Block() as blk:
        @blk.gpsimd
        def _(gp):
            gp.wait_ge(s_cc, T)
            for sm in all_sems:
                gp.sem_clear(sm)

    for cm in reversed(ctx_tensors):
        cm.__exit__(None, None, None)
    return nc


def _prep_in_maps(inputs, T=S):
    """Host-side sharding: per-core input dicts."""
    x = np.ascontiguousarray(np.asarray(inputs["x"], np.float32)[:, :T, :])
    W_f = np.asarray(inputs["W_f"], np.float32)
    W_i = np.asarray(inputs["W_i"], np.float32)
    W_g = np.asarray(inputs["W_g"], np.float32)
    W_o = np.asarray(inputs["W_o"], np.float32)
    b_f = np.asarray(inputs["b_f"], np.float32)
    b_i = np.asarray(inputs["b_i"], np.float32)
    b_g = np.asarray(inputs["b_g"], np.float32)
    b_o = np.asarray(inputs["b_o"], np.float32)
    W_fc = np.ascontiguousarray(np.asarray(inputs["W_fc"], np.float32))
    b_fc = np.asarray(inputs["b_fc"], np.float32)

    xT = np.ascontiguousarray(x.transpose(2, 1, 0))  # [DIN, T, B]
    ones = np.ones((1, 256), np.float32)
    eye = np.eye(128, dtype=np.float32)
    in_maps = []
    for c in range(NCORES):
        sl = slice(LOCH * c, LOCH * (c + 1))
        Wcat = np.concatenate(
            [W_f[:, sl], W_i[:, sl], W_o[:, sl], W_g[:, sl]], axis=1)  # f|i|o|g
        Wx_c = np.ascontiguousarray(Wcat[:DIN])
        Wh_c = np.ascontiguousarray(Wcat[DIN:])
        b_c = np.concatenate([b_f[sl], b_i[sl], b_o[sl], b_g[sl]])[None, :]
        in_maps.append({
            "xT": xT,
            "wxin": Wx_c,
            "whin": Wh_c,
            "wfcin": W_fc,
            "bin": np.ascontiguousarray(b_c),
            "bfcin": np.ascontiguousarray(b_fc[None, :]),
            "onesin": ones,
            "identin": eye,
            "zeroin": np.zeros((128, 256), np.float32),
        })
    return in_maps


def _assemble(results, T=S):
    TWIN = T // NCORES
    out = np.empty((B, T, DOUT), np.float32)
    for c in range(NCORES):
        oT = np.asarray(results[c]["outT"], np.float32)      # [DOUT, TWIN*B]
        blk = oT.reshape(DOUT, TWIN, B).transpose(2, 1, 0)   # [B, TWIN, DOUT]
        out[:, TWIN * c:TWIN * (c + 1), :] = blk
    return out


def get_nc(T=S):
    if T not in _cache:
        _cache[T] = _build_nc(T)
    return _cache[T]


def kernel(**inputs):
    from concourse import bass_utils
    nc = get_nc(S)
    in_maps = _prep_in_maps(inputs, S)
    res = bass_utils.run_bass_kernel_spmd(nc, in_maps, core_ids=list(range(NCORES)))
    return _assemble(res.results, S)



# revision 40
# speedup vs baseline: 1.9822x; 1.9822x over previous
"""Trainium2 Bass kernel for nn_CustomLSTM: B=32, S=512, D_in=512, D_h=1024, D_out=512.

Strategy: 8-way tensor-parallel over the hidden/gate dim. Core c owns 128 h-dims
(block c) and the 4x128 = 512 gate columns that produce them. Per step:
  - PE: gates_preact[32,512] = bias + x_t @ Wx_loc + h_t @ Wh_loc  (fp32r, psum)
  - ACT: sigmoid(f,i,o) + tanh(g)  -> SBUF (untransposed [32,512])
  - PE: 4 transposes -> gates^T [128,32] each (psum)
  - DVE: c' = f*c + i*g ; h^T = o * tanh(c')  (transposed [128,32])
  - h^T tile [128,32] pushed to every core's SBUF with remote_dma_broadcast
    (SWDGE peer-to-peer DMA + remote semaphore) -- no DRAM bounce, no ncfw
    collective.  Each sender lands in slot `pid` of the receivers' gather
    buffer (register AP), so after all 8 broadcasts land every core holds the
    full h_{t+1}^T [1024,32].
The FC output layer runs inside the loop: core c owns output dims
[64c, 64c+64) for ALL timesteps; each step adds 9 small matmuls vs the
gathered h_t (also keeps the PE HAM-warm during the gather window). Results
stream to DRAM in 8-step chunks.
"""

import sys

if "/opt/trn_rl_repo" not in sys.path:
    sys.path.insert(0, "/opt/trn_rl_repo")

import numpy as np

B, S, DIN, DH, DOUT = 32, 512, 512, 1024, 512
NCORES = 8
LOCH = DH // NCORES          # 128 h-dims per core
LOCG = 4 * LOCH              # 512 gate cols per core (f|i|o|g)
LOCO = DOUT // NCORES        # 64 fc out-dims per core
KX = DIN // 128              # 4 x k-tiles
KH = 8                       # 8 h slot tiles
XT_RING = 8                  # xt prefetch ring depth (steps)
XT_AHEAD = 6                 # prefetch distance
FC_RING = 8                  # fc out staging ring (steps)

_cache = {}


def _build_nc(T):
    """Build the SPMD bass program for a T-step LSTM (T divisible by 8)."""
    from concourse import bass
    import concourse.mybir as mybir

    assert T % FC_RING == 0
    dt = mybir.dt
    f32 = dt.float32
    f32r = dt.float32r
    bf16 = dt.bfloat16
    AF = mybir.ActivationFunctionType

    nc = bass.Bass(target_bir_lowering=False, num_devices=NCORES)
    # The kernel is cross-core dependent (peer-to-peer SBUF DMA): the 8 core
    # programs must run concurrently, like a collective.
    nc.has_collectives = True

    # ---------------- I/O ----------------
    xT = nc.dram_tensor("xT", [DIN, T, B], f32, kind="ExternalInput")
    wxin = nc.dram_tensor("wxin", [DIN, LOCG], f32, kind="ExternalInput")
    whin = nc.dram_tensor("whin", [DH, LOCG], bf16, kind="ExternalInput")
    wfcin = nc.dram_tensor("wfcin", [DH, LOCO], bf16, kind="ExternalInput")
    bin_ = nc.dram_tensor("bin", [1, LOCG], f32, kind="ExternalInput")
    bfcin = nc.dram_tensor("bfcin", [1, LOCO], f32, kind="ExternalInput")
    onesin = nc.dram_tensor("onesin", [1, 256], f32, kind="ExternalInput")
    identin = nc.dram_tensor("identin", [128, 128], f32, kind="ExternalInput")
    # out dims [64c, 64c+64) for all (b, t): row b, col t*64+j
    outF = nc.dram_tensor("outF", [B, T * LOCO], f32, kind="ExternalOutput")

    # ---------------- semaphores ----------------
    s_ld = nc.alloc_semaphore("s_ld")        # prologue dma loads (+16)
    s_xt = nc.alloc_semaphore("s_xt")        # xt prefetch dmas (+16)
    s_xtg = nc.alloc_semaphore("s_xtg")      # xt groups confirmed (+1)
    s_mm = nc.alloc_semaphore("s_mm")        # PE last gate-MM (+1/step)
    s_act = nc.alloc_semaphore("s_act")      # ACT sig+tanh done (+1/step)
    s_T = nc.alloc_semaphore("s_T")          # PE transposes done (+1/step)
    s_cp = nc.alloc_semaphore("s_cp")        # DVE c' written (+1/step)
    s_tc = nc.alloc_semaphore("s_tc")        # ACT tanh(c) written (+1/step)
    s_hrdy = nc.alloc_semaphore("s_hrdy")    # DVE h^T written (+1/step)
    s_dv1 = nc.alloc_semaphore("s_dv1")      # DVE writeback fences (+2/step)
    s_fcm = nc.alloc_semaphore("s_fcm")      # PE last FC-MM (+1/step)
    s_fce = nc.alloc_semaphore("s_fce")      # DVE FC evac (+1/step)
    s_fout = nc.alloc_semaphore("s_fout")    # FC out dmas (+16 per chunk)
    # Cross-core h^T arrival, PARITY-SPLIT: round r (the broadcast after DVE
    # step r, carrying h_{r+1}) increments s_har[r%2]; +2 per sender -> +16
    # per round. Parity split makes the count unambiguous: round r+2 incs
    # cannot exist anywhere until every core consumed round r (round r+2
    # needs my round r+1 send, which needs my PE step r+1, which waited for
    # round r), so a fast peer's next-round increments can never substitute
    # for a slow peer's missing ones at the same threshold. Cleared ONLY in
    # the tail (after quiescence), never in the prologue: a prologue clear
    # could race with a fast peer's round-0 packets.
    s_har = [nc.alloc_semaphore("s_har0"), nc.alloc_semaphore("s_har1")]
    s_sent = nc.alloc_semaphore("s_sent")    # local send-complete (+16/round)
    s_hfree = nc.alloc_semaphore("s_hfree")  # Pool relay: hst slot reusable (+1)
    pre_sems = [s_ld, s_xt, s_xtg, s_mm, s_act, s_T, s_cp, s_tc, s_hrdy,
                s_dv1, s_fcm, s_fce, s_fout, s_hfree]
    all_sems = pre_sems + s_har + [s_sent]

    def har_wait(t):
        """(sem, value) guaranteeing rounds 0..t-1 have fully landed."""
        return s_har[(t - 1) % 2], 16 * ((t + 1) // 2)

    # ---------------- on-chip tensors ----------------
    ctx_tensors = []

    def sbuf(name, shape, dtype=f32):
        cm = nc.sbuf_tensor(name, shape, dtype)
        t = cm.__enter__()
        ctx_tensors.append(cm)
        return t

    def psum(name, shape, dtype=f32):
        cm = nc.psum_tensor(name, shape, dtype)
        t = cm.__enter__()
        ctx_tensors.append(cm)
        return t

    wx = sbuf("wx", [128, KX * LOCG])          # x-weights, tile d at cols d*512
    wh = sbuf("wh", [128, KH * LOCG], bf16)    # h-weights, slot tile m at m*512
    wfc = sbuf("wfc", [128, KH * LOCO], bf16)  # fc weights, tile m at m*64
    xt = sbuf("xt", [128, XT_RING * KX * B])   # x_t^T ring: step r at cols r*128
    # gathered h^T: bf16 payload viewed through an f32 tensor (the broadcast
    # ISA encoding only round-trips natural f32 APs). 2 parities x 8 slots x
    # HB f32 cols (= 32 bf16 each).
    gath = sbuf("gath", [128, 2 * 128])
    actsb = sbuf("actsb", [128, 2 * LOCG])     # activated gates (rows 0:32)
    csb = sbuf("csb", [128, 2 * B])            # c^T state, 2 parities
    t1b = sbuf("t1b", [128, B])
    t2b = sbuf("t2b", [128, B])
    gsb = sbuf("gsb", [128, B])
    tcb = sbuf("tcb", [128, B])
    HB = B // 2                                # f32 cols per bf16 h-slice
    hst = sbuf("hst", [128, 2 * HB])           # h^T staging (bf16-as-f32)
    fcring = sbuf("fcring", [128, FC_RING * LOCO])  # fc out staging (rows 0:32)
    bvec = sbuf("bvec", [1, LOCG])
    bfcv = sbuf("bfcv", [1, LOCO])
    onesb = sbuf("onesb", [1, 256])
    ident = sbuf("ident", [128, 128])

    pg = psum("pg", [128, 2 * 512])            # gate psum, 2 banks (rows 0:32)
    pt = psum("pt", [128, 2 * 512])            # transposed gates, 2 banks
    pfc = psum("pfc", [128, 2 * 512])          # fc psum, 2 banks (cols 0:64)

    def r(ap):
        return ap.bitcast(f32r)

    # ================= Block P0: clear local sems =================
    with nc.Block() as blk:
        @blk.gpsimd
        def _(gp):
            for sm in pre_sems:
                gp.sem_clear(sm)

    # ================= Block P1: load weights / init state =================
    with nc.Block() as blk:
        @blk.sync
        def _(sp):
            n = 0
            for d in range(KX):
                sp.dma_start(r(wx[:, 512 * d:512 * (d + 1)]),
                             r(wxin[128 * d:128 * (d + 1), :])).then_inc(s_ld, 16)
                n += 1
            for m in range(KH):
                sp.dma_start(wh[:, 512 * m:512 * (m + 1)],
                             whin[128 * m:128 * (m + 1), :]).then_inc(s_ld, 16)
                n += 1
            for m in range(KH):
                sp.dma_start(wfc[:, 64 * m:64 * (m + 1)],
                             wfcin[128 * m:128 * (m + 1), :]).then_inc(s_ld, 16)
                n += 1
            P = min(XT_AHEAD, T)
            for t0 in range(P):
                for d in range(KX):
                    sp.dma_start(r(xt[:, 128 * t0 + 32 * d:128 * t0 + 32 * (d + 1)]),
                                 r(xT[128 * d:128 * (d + 1), t0, :])).then_inc(s_xt, 16)
            sp.wait_ge(s_xt, 16 * KX * P)
            sp.sem_inc(s_xtg, P)
            sp.dma_start(r(bvec[0:1, :]), r(bin_[0:1, :])).then_inc(s_ld, 16); n += 1
            sp.dma_start(r(bfcv[0:1, :]), r(bfcin[0:1, :])).then_inc(s_ld, 16); n += 1
            sp.dma_start(r(onesb[0:1, :]), r(onesin[0:1, :])).then_inc(s_ld, 16); n += 1
            sp.dma_start(ident[:, :], identin[:, :]).then_inc(s_ld, 16); n += 1
            sp.wait_ge(s_ld, 16 * n)

        @blk.vector
        def _(ve):
            ve.memset(csb[:, 0:B], 0.0)          # c_0 = 0
            ve.memset(gath[:, 0:128], 0.0)       # h_0 = 0 (parity 0)

    # ================= Block M: the recurrence =================
    with nc.Block() as blk:
        @blk.tensor
        def _(te):
            for t in range(T):
                pi = t % 2
                pgb = pg[0:32, 512 * pi:512 * (pi + 1)]        # gate psum bank
                if t >= 2:
                    te.wait_ge(s_act, t - 1)                   # pg[pi] free
                te.wait_ge(s_xtg, t + 1)                       # xt(t) loaded
                te.matmul(pgb, r(onesb[0:1, 0:32]), r(bvec[0:1, :]),
                          start=True, stop=False)
                xs = 128 * (t % XT_RING)
                for d in range(KX):
                    te.matmul(pgb, r(xt[:, xs + 32 * d:xs + 32 * (d + 1)]),
                              r(wx[:, 512 * d:512 * (d + 1)]),
                              start=False, stop=False)
                if t >= 1:
                    te.wait_ge(*har_wait(t))                   # h_t gathered
                gb = 128 * pi
                for m in range(KH):
                    mm = te.matmul(pgb,
                                   gath[:, gb + 16 * m:gb + 16 * (m + 1)].bitcast(
                                       bf16),
                                   wh[:, 512 * m:512 * (m + 1)],
                                   start=False, stop=(m == KH - 1))
                    if m == KH - 1:
                        mm.then_inc(s_mm, 1)
                # FC for output index j = t-1: out_j = fc(h_{j+1}) = fc(h_t),
                # using the same gathered tiles as the h-MMs above (fills the
                # PE bubble while ACT runs). Output T-1 runs after the loop.
                if t >= 1:
                    j = t - 1
                    pfcb = pfc[0:32, 512 * (j % 2):512 * (j % 2) + LOCO]
                    if j >= 2:
                        te.wait_ge(s_fce, j - 1)               # pfc[j%2] free
                    te.matmul(pfcb, r(onesb[0:1, 0:32]), r(bfcv[0:1, :]),
                              start=True, stop=False)
                    for m in range(KH):
                        mm = te.matmul(
                            pfcb,
                            gath[:, gb + 16 * m:gb + 16 * (m + 1)].bitcast(bf16),
                            wfc[:, 64 * m:64 * (m + 1)],
                            start=False, stop=(m == KH - 1))
                        if m == KH - 1:
                            mm.then_inc(s_fcm, 1)
                # transposes of activated gates -> pt[pi] cols 0:128 (F|I|O|G)
                te.wait_ge(s_act, t + 1)
                ab = 512 * pi
                ptb = 512 * pi
                for gi in range(4):
                    mm = te.matmul(pt[:, ptb + 32 * gi:ptb + 32 * (gi + 1)],
                                   actsb[0:32, ab + 128 * gi:ab + 128 * (gi + 1)],
                                   ident[0:32, 0:32],
                                   is_transpose=True, start=True, stop=True,
                                   skip_group_check=True)
                    if gi == 3:
                        mm.then_inc(s_T, 1)
            # epilogue: FC for output index T-1 (h_T, gathered by round T-1)
            j = T - 1
            te.wait_ge(s_har[(T - 1) % 2], 16 * ((T + 1) // 2))
            pfcb = pfc[0:32, 512 * (j % 2):512 * (j % 2) + LOCO]
            te.wait_ge(s_fce, j - 1)
            te.matmul(pfcb, r(onesb[0:1, 0:32]), r(bfcv[0:1, :]),
                      start=True, stop=False)
            gb = 128 * (T % 2)
            for m in range(KH):
                mm = te.matmul(pfcb,
                               gath[:, gb + 16 * m:gb + 16 * (m + 1)].bitcast(bf16),
                               wfc[:, 64 * m:64 * (m + 1)],
                               start=False, stop=(m == KH - 1))
                if m == KH - 1:
                    mm.then_inc(s_fcm, 1)

        @blk.scalar
        def _(ac):
            for t in range(T):
                pi = t % 2
                ab = 512 * pi
                ac.wait_ge(s_mm, t + 1)
                if t >= 2:
                    ac.wait_ge(s_T, t - 1)                     # actsb[pi] free
                ac.activation(actsb[0:32, ab:ab + 384],
                              pg[0:32, 512 * pi:512 * pi + 384], AF.Sigmoid)
                ac.activation(actsb[0:32, ab + 384:ab + 512],
                              pg[0:32, 512 * pi + 384:512 * pi + 512],
                              AF.Tanh).then_inc(s_act, 1)
                ac.wait_ge(s_cp, t + 1)
                if t >= 1:
                    ac.wait_ge(s_hrdy, t)                      # tcb free
                cpb = ((t + 1) % 2) * B
                ac.activation(tcb[:, 0:B], csb[:, cpb:cpb + B],
                              AF.Tanh).then_inc(s_tc, 1)

        @blk.vector
        def _(ve):
            for t in range(T):
                pi = t % 2
                po = (t + 1) % 2
                ptb = 512 * pi
                F = pt[:, ptb + 0:ptb + 32]
                I = pt[:, ptb + 32:ptb + 64]
                O = pt[:, ptb + 64:ptb + 96]
                G = pt[:, ptb + 96:ptb + 128]
                ve.wait_ge(s_T, t + 1)
                ve.tensor_copy(gsb[:, :], G).then_inc(s_dv1, 1)  # PSUM -> SBUF
                ve.tensor_mul(t1b[:, :], F, csb[:, pi * B:pi * B + B])
                ve.wait_ge(s_dv1, 2 * t + 1)                   # gsb writeback fence
                ve.tensor_mul(t2b[:, :], I, gsb[:, :]).then_inc(s_dv1, 1)
                if t >= 2:
                    ve.wait_ge(s_tc, t - 1)                    # csb[po] free
                ve.wait_ge(s_dv1, 2 * t + 2)                   # t1/t2 writeback fence
                ve.tensor_add(csb[:, po * B:po * B + B], t1b[:, :],
                              t2b[:, :]).then_inc(s_cp, 1)
                ve.wait_ge(s_tc, t + 1)
                if t >= 2:
                    ve.wait_ge(s_hfree, t - 1)                 # hst[pi] send done
                ve.tensor_mul(hst[:, pi * HB:pi * HB + HB].bitcast(bf16), O,
                              tcb[:, 0:B]).then_inc(s_hrdy, 1)
                # FC evac psum -> staging ring (output index j = t-1)
                if t >= 1:
                    j = t - 1
                    ve.wait_ge(s_fcm, j + 1)
                    if j >= FC_RING:
                        ve.wait_ge(s_fout, 16 * (j // FC_RING))  # ring slot free
                    ve.tensor_copy(fcring[0:32, LOCO * (j % FC_RING):
                                           LOCO * (j % FC_RING + 1)],
                                   pfc[0:32, 512 * (j % 2):
                                       512 * (j % 2) + LOCO]).then_inc(s_fce, 1)
            # epilogue evac for output index T-1
            j = T - 1
            ve.wait_ge(s_fcm, j + 1)
            ve.wait_ge(s_fout, 16 * (j // FC_RING))
            ve.tensor_copy(fcring[0:32, LOCO * (j % FC_RING):
                                   LOCO * (j % FC_RING + 1)],
                           pfc[0:32, 512 * (j % 2):
                               512 * (j % 2) + LOCO]).then_inc(s_fce, 1)

        @blk.sync
        def _(sp):
            nxt = 16 * KX * min(XT_AHEAD, T)
            for t in range(T):
                tf = t + XT_AHEAD
                if tf < T:
                    if t >= 2:
                        sp.wait_ge(s_mm, t - 1)                # ring slot free
                    xs = 128 * (tf % XT_RING)
                    for d in range(KX):
                        sp.dma_start(r(xt[:, xs + 32 * d:xs + 32 * (d + 1)]),
                                     r(xT[128 * d:128 * (d + 1), tf, :])).then_inc(
                                         s_xt, 16)
                    nxt += 16 * KX
                    sp.wait_ge(s_xt, nxt)
                    sp.sem_inc(s_xtg, 1)
                # output chunk q covers output indices [8q, 8q+8); index j is
                # evac'd during step j+1, so chunk q completes at step 8q+8
                if t % FC_RING == 0 and t >= FC_RING:
                    q = t // FC_RING - 1
                    sp.wait_ge(s_fce, FC_RING * (q + 1))
                    sp.dma_start(outF[:, LOCO * FC_RING * q:
                                      LOCO * FC_RING * (q + 1)],
                                 fcring[0:32, :]).then_inc(s_fout, 16)
                    sp.wait_ge(s_fout, 16 * (q + 1))
            # final output chunk (indices [T-8, T))
            q = T // FC_RING - 1
            sp.wait_ge(s_fce, T)
            sp.dma_start(outF[:, LOCO * FC_RING * q:LOCO * FC_RING * (q + 1)],
                         fcring[0:32, :]).then_inc(s_fout, 16)
            sp.wait_ge(s_fout, 16 * (q + 1))

        @blk.gpsimd
        def _(gp):
            from concourse import library_config
            gp.load_library(library_config.remote_dma)
            pidreg = gp.alloc_register("pidreg")
            gp.reg_load(pidreg, nc.partition_id_tensor[0:1, 0:1])
            RD = [(0, k) for k in range(NCORES)]

            def prep(round_t):
                """Queue the round-t broadcast: hst parity round_t%2 into
                gath parity (round_t+1)%2, slot = this core's rank. The out
                AP must be compile-time static (the broadcast ISA encoding
                has no register-AP variant), so emit an If-chain with one
                statically-specialized prep per rank."""
                ps = (round_t % 2) * HB
                gbase = ((round_t + 1) % 2) * 128
                for k in range(NCORES):
                    with gp.If_eq(pidreg, k):
                        gp.remote_dma_broadcast(
                            gath[:, gbase + k * HB:gbase + (k + 1) * HB],
                            hst[:, ps:ps + HB],
                            s_har[round_t % 2], s_sent, rdests=RD)

            prep(0)
            for t in range(T):
                if t >= 1:
                    # Mirror the PE's arrival wait (always already satisfied
                    # here) so the round-t broadcast carries "rounds < t
                    # landed at my core" transitively to every peer -- the
                    # ordering the next-round senders rely on.
                    gp.wait_ge(*har_wait(t))
                gp.wait_ge(s_hrdy, t + 1)
                gp.trigger_dma(1)
                # Serialize: send t fully flushed before anything else. Keeps
                # every s_sent wait at the max-possible value and lets the
                # plain relay sem stand in for it on the DVE.
                gp.wait_ge(s_sent, 16 * (t + 1))
                gp.sem_inc(s_hfree, 1)                         # hst[t%2] reusable
                if t + 1 < T:
                    prep(t + 1)
            gp.wait_ge(s_har[0], 16 * ((T + 1) // 2))          # all arrivals seen
            gp.wait_ge(s_har[1], 16 * (T // 2))

    # ================= Block E: final cleanup =================
    with nc.Block() as blk:
        @blk.gpsimd
        def _(gp):
            # s_harr/s_sent are exactly 16*T here (waits above), and no
            # further remote increments can exist -> safe to clear.
            for sm in all_sems:
                gp.sem_clear(sm)

    for cm in reversed(ctx_tensors):
        cm.__exit__(None, None, None)
    # Lower extended-ISA instructions (remote-DMA descs, trigger, library
    # load) to their 64-byte encodings. The plain bass.Bass serialization
    # path doesn't run this Bacc pass, and walrus rejects the empty
    # encodings with "ISA wrong length".
    mybir.codegen_inst_isa_subclasses(nc)
    return nc


def _prep_in_maps(inputs, T=S):
    """Host-side sharding: per-core input dicts."""
    x = np.ascontiguousarray(np.asarray(inputs["x"], np.float32)[:, :T, :])
    W_f = np.asarray(inputs["W_f"], np.float32)
    W_i = np.asarray(inputs["W_i"], np.float32)
    W_g = np.asarray(inputs["W_g"], np.float32)
    W_o = np.asarray(inputs["W_o"], np.float32)
    b_f = np.asarray(inputs["b_f"], np.float32)
    b_i = np.asarray(inputs["b_i"], np.float32)
    b_g = np.asarray(inputs["b_g"], np.float32)
    b_o = np.asarray(inputs["b_o"], np.float32)
    W_fc = np.ascontiguousarray(np.asarray(inputs["W_fc"], np.float32))
    b_fc = np.asarray(inputs["b_fc"], np.float32)

    import ml_dtypes
    bf16 = ml_dtypes.bfloat16
    xT = np.ascontiguousarray(x.transpose(2, 1, 0))  # [DIN, T, B]
    ones = np.ones((1, 256), np.float32)
    eye = np.eye(128, dtype=np.float32)
    in_maps = []
    for c in range(NCORES):
        sl = slice(LOCH * c, LOCH * (c + 1))
        so = slice(LOCO * c, LOCO * (c + 1))
        Wcat = np.concatenate(
            [W_f[:, sl], W_i[:, sl], W_o[:, sl], W_g[:, sl]], axis=1)  # f|i|o|g
        Wx_c = np.ascontiguousarray(Wcat[:DIN])
        Wh_c = np.ascontiguousarray(Wcat[DIN:]).astype(bf16)
        b_c = np.concatenate([b_f[sl], b_i[sl], b_o[sl], b_g[sl]])[None, :]
        in_maps.append({
            "xT": xT,
            "wxin": Wx_c,
            "whin": Wh_c,
            "wfcin": np.ascontiguousarray(W_fc[:, so]).astype(bf16),
            "bin": np.ascontiguousarray(b_c),
            "bfcin": np.ascontiguousarray(b_fc[None, so]),
            "onesin": ones,
            "identin": eye,
        })
    return in_maps


def _assemble(results, T=S):
    out = np.empty((B, T, DOUT), np.float32)
    for c in range(NCORES):
        blk = np.asarray(results[c]["outF"], np.float32).reshape(B, T, LOCO)
        out[:, :, LOCO * c:LOCO * (c + 1)] = blk
    return out


def get_nc(T=S):
    if T not in _cache:
        _cache[T] = _build_nc(T)
    return _cache[T]


def kernel(**inputs):
    from concourse import bass_utils
    nc = get_nc(S)
    in_maps = _prep_in_maps(inputs, S)
    res = bass_utils.run_bass_kernel_spmd(nc, in_maps, core_ids=list(range(NCORES)))
    return _assemble(res.results, S)
